# revision 12
# baseline (speedup 1.0000x reference)
"""Trainium2 Bass kernel for nn_Decoder (LSTM decoder + additive attention + vocab head).

Sharding (8 NeuronCores, SPMD — identical program, per-core data):
  - LSTM h/c recurrence replicated on all cores, transposed layout (units on
    partitions, batch on free). Hidden state stored scaled H=2h so sigmoid
    gates fuse into scalar_tensor_tensor ops (0.5 folded into W_hh/Wa/Wc-h).
  - Attention + s_t = tanh(Wc @ [ctx; h]) sharded over batch (4/core) via a
    host-side batch permutation. Scores computed transposed ([128 (b,l), 4 t])
    with the padding mask + va_b folded into the Exp bias; softmax division
    deferred until after the enc contraction (per-partition rescale).
  - s gathered incrementally with 8 small AllGathers (overlap the recurrence);
    vocab projection sharded over V (4000 rows/core), chunks interleaved
    2-per-LSTM-step so the PE never idles. Wh_b is added on the host; the
    device writes bf16 logits.
Only Tanh/Exp ACT funcs are used (single table set).
"""

import numpy as np
import ml_dtypes

V, E, H, LQ, B = 32000, 256, 512, 32, 32
NCORES = 8
VS = V // NCORES      # 4000
VCH = 500             # vocab chunk (fp32 psum bank = 512)
NVC = VS // VCH       # 8
NBLK = LQ // 4        # 8 four-step attention blocks

_cache = {}


def _build_program():
    import concourse.bass as bass
    import concourse.mybir as mybir
    import concourse.tile as tile
    from concourse import bacc

    f32 = mybir.dt.float32
    bf16 = mybir.dt.bfloat16
    i32 = mybir.dt.int32

    nc = bacc.Bacc("TRN2", target_bir_lowering=False, debug=False,
                   num_devices=NCORES)

    d = {}

    def inp(name, shape, dtype):
        d[name] = nc.dram_tensor(name, shape, dtype, kind="ExternalInput").ap()

    inp("emb", [V, E], f32)
    inp("idx", [128, 8], i32)
    inp("whh", [128, 64 * 128], bf16)
    inp("wih", [128, 32 * 128], bf16)
    inp("wa", [128, 16 * 128], bf16)
    inp("wc", [128, 32 * 128], bf16)
    inp("ua", [128, 16 * 128], bf16)
    inp("bias_units", [128, 16], f32)
    inp("bias_kp", [128, 4], f32)
    inp("va", [128, 4], bf16)
    inp("wcb_row", [1, 512], bf16)
    inp("h0t", [128, 128], bf16)
    inp("encr", [128, 512], bf16)
    inp("enct", [128, 512], bf16)
    inp("maskbias", [128, 1], f32)
    inp("m16", [16, 4], f32)
    inp("bd16", [128, 16], bf16)
    inp("bdm", [128, 4], bf16)
    inp("wht", [512, VS], bf16)
    d["out"] = nc.dram_tensor("out", [B, LQ, VS], bf16, kind="ExternalOutput").ap()
    d["ag_in"] = [nc.dram_tensor(f"agi{a}", [128, 64], bf16).ap()
                  for a in range(NBLK)]
    d["ag_out"] = [nc.dram_tensor(f"ago{a}", [128 * NCORES, 64], bf16,
                                  addr_space="Shared").ap() for a in range(NBLK)]

    with tile.TileContext(nc) as tc:
        _emit(tc, bass, mybir, d)
    nc.compile()
    return nc


def _emit(tc, bass, mybir, d):
    from concourse.masks import make_identity

    f32 = mybir.dt.float32
    bf16 = mybir.dt.bfloat16
    i32 = mybir.dt.int32
    AF = mybir.ActivationFunctionType
    OP = mybir.AluOpType
    nc = tc.nc

    perm = tc.alloc_tile_pool(name="perm", bufs=1)
    est = tc.alloc_tile_pool(name="est", bufs=2)
    big = tc.alloc_tile_pool(name="big", bufs=1)
    psG = tc.alloc_tile_pool(name="psG", bufs=1, space="PSUM")
    psA = tc.alloc_tile_pool(name="psA", bufs=3, space="PSUM")
    psV = tc.alloc_tile_pool(name="psV", bufs=3, space="PSUM")
    outp = tc.alloc_tile_pool(name="outp", bufs=3)

    def pt(name, shape, dtype):
        return perm.tile(shape, dtype, tag=name, name=name)

    # ---- persistent SBUF ----------------------------------------------
    whh = pt("whh", [128, 64 * 128], bf16)   # lhsT tile (kt,mt) @ (kt*16+mt)*128
    wih = pt("wih", [128, 32 * 128], bf16)   # (kt*16+mt)*128, kt<2
    wa = pt("wa", [128, 16 * 128], bf16)     # (kt*4+mt)*128
    wc = pt("wc", [128, 32 * 128], bf16)     # (kt*4+mt)*128, kt<8
    ua = pt("ua", [128, 16 * 128], bf16)
    bias_u = pt("bias_u", [128, 16], f32)
    bias_kp = pt("bias_kp", [128, 4], f32)
    va_sb = pt("va_sb", [128, 4], bf16)
    wcb_sb = pt("wcb_sb", [1, 512], bf16)
    idx_sb = pt("idx_sb", [128, 8], i32)
    encr = pt("encr", [128, 512], bf16)      # rows (b_own,l), cols u
    enct = pt("enct", [128, 512], bf16)      # cols (kt, (b_own,l))
    mbias = pt("mbias", [128, 1], f32)       # va_b + (0 | -30) per (b_own,l)
    m16_sb = pt("m16_sb", [16, 4], f32)
    bd16_sb = pt("bd16_sb", [128, 16], bf16)
    bdm = pt("bdm", [128, 4], bf16)
    hist = pt("hist", [128, 33 * 128], bf16)  # col = slot*128 + kt*32 + b; H=2h
    cS = pt("cS", [128, 128], f32)            # S = 2c
    xpb = pt("xpb", [128, 16 * 1024], bf16)  # col = mt*1024 + t*32 + b
    xT = pt("xT", [128, 2 * 1024], bf16)     # col = kt*1024 + (t*32+b)
    kp = pt("kp", [128, 512], f32)           # col = mt*128 + b*32 + l
    sT = pt("sT", [128, 512], bf16)           # col = mt*128 + a*16 + (b*4+t)
    wh_sb = pt("wh_sb", [128, 4 * VS], bf16)  # col = kt*VS + v
    ident = pt("ident", [128, 128], f32)
    ident_b = pt("ident_b", [128, 128], bf16)
    ones_b = pt("ones_b", [1, 128], bf16)
    sg = pt("sg", [128, NBLK * 4 * 128], bf16)  # col = a*512+kt*128+(j2*16+b*4+t)

    hist_v = hist[:].rearrange("p (s k b) -> p s k b", s=33, k=4)
    xpb_v = xpb[:].rearrange("p (m t) -> p m t", m=16)
    kp_v = kp[:].rearrange("p (m b l) -> p m b l", m=4, b=4)
    sT_v = sT[:].rearrange("p (k a i) -> p k a i", k=4, a=NBLK)
    sg_v = sg[:].rearrange("p (a k i) -> p a k i", a=NBLK, k=4)
    sg_v5 = sg[:].rearrange("p (a k j i) -> p a k j i", a=NBLK, k=4, j=NCORES)

    # ---- load weights/constants ---------------------------------------
    # step-0-critical loads first; the big vocab weights (needed from t=8)
    # last so their transfers don't delay the recurrence start
    for dst, src in ((idx_sb, "idx"), (wih, "wih"), (whh, "whh"),
                     (bias_u, "bias_units"), (ua, "ua"), (enct, "enct"),
                     (bias_kp, "bias_kp"), (wa, "wa"), (wc, "wc"),
                     (encr, "encr"), (va_sb, "va"), (wcb_sb, "wcb_row"),
                     (mbias, "maskbias"), (m16_sb, "m16"),
                     (bd16_sb, "bd16"), (bdm, "bdm")):
        nc.sync.dma_start(dst[:], d[src][:])
    nc.sync.dma_start(hist[:, 0:128], d["h0t"][:])
    for kt in range(4):
        nc.sync.dma_start(wh_sb[:, kt * VS:(kt + 1) * VS],
                          d["wht"][kt * 128:(kt + 1) * 128, :])

    make_identity(nc, ident[:])
    make_identity(nc, ident_b[:])
    nc.gpsimd.memset(ones_b[:], 1.0)
    nc.gpsimd.memset(cS[:], 0.0)

    # ---- embedding gather + transpose ---------------------------------
    xrow_all = big.tile([128, 8, 256], f32, tag="xrow", name="xrow")
    nc.gpsimd.indirect_dma_start(
        out=xrow_all[:], out_offset=None, in_=d["emb"][:],
        in_offset=bass.IndirectOffsetOnAxis(ap=idx_sb[:, 0:8], axis=0))

    def emb_transpose(c):
        for kt in range(2):
            tp = psA.tile([128, 128], f32, tag="a", name="tpx")
            nc.tensor.transpose(tp[:], xrow_all[:, c, kt * 128:(kt + 1) * 128],
                                ident[:, 0:128])
            seg = xT[:, kt * 1024 + c * 128: kt * 1024 + (c + 1) * 128]
            if (c + kt) % 2 == 0:
                nc.vector.tensor_copy(out=seg, in_=tp[:])
            else:
                nc.scalar.activation(seg, tp[:], AF.Copy)

    # ---- x-projection xpb = W_ih @ x^T + (b_ih + b_hh), bf16 ----------
    def xproj(mt, h2):
        ps = psV.tile([128, 512], f32, tag="v", name="vps")
        for kt in range(2):
            nc.tensor.matmul(
                ps[:],
                wih[:, (kt * 16 + mt) * 128:(kt * 16 + mt + 1) * 128],
                xT[:, kt * 1024 + h2 * 512: kt * 1024 + (h2 + 1) * 512],
                start=(kt == 0), stop=(kt == 1))
        dst = xpb[:, mt * 1024 + h2 * 512: mt * 1024 + (h2 + 1) * 512]
        if (mt + h2) % 2 == 0:
            nc.vector.tensor_scalar(out=dst, in0=ps[:],
                                    scalar1=bias_u[:, mt:mt + 1],
                                    scalar2=None, op0=OP.add)
        else:
            nc.scalar.activation(dst, ps[:], AF.Identity,
                                 bias=bias_u[:, mt:mt + 1])

    # ---- key projection kp = Ua @ enc^T + (Ua_b + Wa_b) ---------------
    def keyproj(mt):
        ps = psA.tile([128, 128], f32, tag="a", name="kp_ps")
        for kt in range(4):
            nc.tensor.matmul(
                ps[:], ua[:, (kt * 4 + mt) * 128:(kt * 4 + mt + 1) * 128],
                enct[:, kt * 128:(kt + 1) * 128],
                start=(kt == 0), stop=(kt == 3))
        nc.vector.tensor_scalar(out=kp[:, mt * 128:(mt + 1) * 128], in0=ps[:],
                                scalar1=bias_kp[:, mt:mt + 1], scalar2=None,
                                op0=OP.add)

    # ==== per-step bodies ==============================================
    def lstm_step(t):
        # Gfg = [f-gates | g-gates] (mt 4..11); Gio = [i | o] (mt 0..3,12..15)
        Gfg = psG.tile([128, 256], f32, tag="gfg", name="Gfg")
        Gio = psG.tile([128, 256], f32, tag="gio", name="Gio")
        nc.tensor.matmul(Gfg[:], ident_b[:], xpb_v[:, 4:12, t * 32:(t + 1) * 32],
                         start=True, stop=False, skip_group_check=True)
        nc.tensor.matmul(Gio[:, 0:128], ident_b[:],
                         xpb_v[:, 0:4, t * 32:(t + 1) * 32],
                         start=True, stop=False, skip_group_check=True)
        nc.tensor.matmul(Gio[:, 128:256], ident_b[:],
                         xpb_v[:, 12:16, t * 32:(t + 1) * 32],
                         start=True, stop=False, skip_group_check=True)

        def gmm(dst, col, mt):
            for kt in range(4):
                nc.tensor.matmul(
                    dst[:, col * 32:(col + 1) * 32],
                    whh[:, (kt * 16 + mt) * 128:(kt * 16 + mt + 1) * 128],
                    hist_v[:, t, kt, :], start=False, stop=(kt == 3),
                    skip_group_check=True)
        for mt in range(4, 12):        # f then g
            gmm(Gfg, mt - 4, mt)
        for mt in range(4):            # i
            gmm(Gio, mt, mt)
        for mt in range(12, 16):       # o
            gmm(Gio, mt - 8, mt)
        tf_ = est.tile([128, 128], f32, tag="tf", name="tf_")
        tg = est.tile([128, 128], f32, tag="tg", name="tg")
        ti = est.tile([128, 128], f32, tag="ti", name="ti")
        to_ = est.tile([128, 128], f32, tag="to", name="to_")
        nc.scalar.activation(tf_[:], Gfg[:, 0:128], AF.Tanh)
        nc.scalar.activation(tg[:], Gfg[:, 128:256], AF.Tanh)
        nc.scalar.activation(ti[:], Gio[:, 0:128], AF.Tanh)
        nc.scalar.activation(to_[:], Gio[:, 128:256], AF.Tanh)
        # S' = 0.5*(t_f+1)*S + (t_i+1)*g   (S = 2c)
        P = est.tile([128, 128], f32, tag="P", name="P")
        Bt = est.tile([128, 128], f32, tag="Bt", name="Bt")
        nc.vector.scalar_tensor_tensor(out=P[:], in0=tf_[:],
                                       scalar=1.0, in1=cS[:],
                                       op0=OP.add, op1=OP.mult)
        nc.vector.scalar_tensor_tensor(out=Bt[:], in0=ti[:],
                                       scalar=1.0, in1=tg[:],
                                       op0=OP.add, op1=OP.mult)
        nc.vector.scalar_tensor_tensor(out=cS[:], in0=P[:], scalar=0.5,
                                       in1=Bt[:], op0=OP.mult, op1=OP.add)
        tct = est.tile([128, 128], f32, tag="tct", name="tct")
        nc.scalar.activation(tct[:], cS[:], AF.Tanh, scale=0.5)
        # H = (t_o+1)*tanh(c) = 2h
        nc.vector.scalar_tensor_tensor(
            out=hist[:, (t + 1) * 128:(t + 2) * 128], in0=to_[:],
            scalar=1.0, in1=tct[:], op0=OP.add, op1=OP.mult)

    attn_state = {}

    def attention_a(a):
        t0 = 4 * a
        qp = psA.tile([128, 4, 4, 4], f32, tag="a", name="qp")  # (mt, t, b)
        for mt in range(4):
            for kt in range(4):
                nc.tensor.matmul(
                    qp[:, mt, :, :],
                    wa[:, (kt * 4 + mt) * 128:(kt * 4 + mt + 1) * 128],
                    hist_v[:, t0 + 1:t0 + 5, kt, 0:4],
                    start=(kt == 0), stop=(kt == 3))
        tin = big.tile([128, 4, 4, 4, 32], f32, tag="tin", name="tin")
        for mt in range(4):
            nc.vector.tensor_tensor(
                out=tin[:, mt],
                in0=qp[:, mt].unsqueeze(3).to_broadcast([128, 4, 4, 32]),
                in1=kp_v[:, mt].unsqueeze(1).to_broadcast([128, 4, 4, 32]),
                op=OP.add)
        tnh = big.tile([128, 2048], bf16, tag="tnh", name="tnh")
        for mt in range(4):
            nc.scalar.activation(
                tnh[:, mt * 512:(mt + 1) * 512],
                tin[:, mt].rearrange("p a b l -> p (a b l)"), AF.Tanh)
        attn_state[a] = dict(tnh=tnh)

    def attention_b(a):
        tnh = attn_state[a]["tnh"]
        # scores transposed: escT[(b,l), t], contracted over u via tnh-as-lhsT
        escT = psA.tile([128, 4], f32, tag="a", name="escT")
        for t in range(4):
            for mt in range(4):
                nc.tensor.matmul(
                    escT[:, t:t + 1],
                    tnh[:, mt * 512 + t * 128: mt * 512 + (t + 1) * 128],
                    va_sb[:, mt:mt + 1],
                    start=(mt == 0), stop=(mt == 3), skip_group_check=True)
        # esc = exp(score + va_b + mask*-30)  (unnormalized)
        esc4 = est.tile([128, 4], bf16, tag="esc4", name="esc4")
        nc.scalar.activation(esc4[:], escT[:], AF.Exp, bias=mbias[:, 0:1])
        # den16[(b,t)] = sum_l esc[(b,l), t]
        den16 = psA.tile([16, 4], f32, tag="a", name="den16")
        nc.tensor.matmul(den16[:], bd16_sb[:], esc4[:], start=True, stop=True)
        rec16 = est.tile([16, 4], f32, tag="rec16", name="rec16")
        nc.vector.reciprocal(rec16[:], den16[:])
        rm = est.tile([16, 4], f32, tag="rm", name="rm")
        nc.vector.tensor_tensor(out=rm[:], in0=rec16[:], in1=m16_sb[:],
                                op=OP.mult)
        rden = est.tile([16, 1], f32, tag="rden", name="rden")
        nc.vector.tensor_reduce(out=rden[:], in_=rm[:],
                                axis=mybir.AxisListType.X, op=OP.add)
        # block-diag weights A[(b,l), (b',t)] = esc[(b,l), t] * [b'==b]
        abig = est.tile([128, 4, 4], bf16, tag="abig", name="abig")
        nc.vector.tensor_tensor(
            out=abig[:], in0=bdm[:].unsqueeze(2).to_broadcast([128, 4, 4]),
            in1=esc4[:].unsqueeze(1).to_broadcast([128, 4, 4]), op=OP.mult)
        cxp = psA.tile([16, 512], f32, tag="a", name="cxp")
        nc.tensor.matmul(cxp[:],
                         abig[:].rearrange("p b t -> p (b t)"),
                         encr[:], start=True, stop=True)
        # normalize deferred: ctx = cxp * (1/den) per partition (b,t)
        cxr = est.tile([16, 512], f32, tag="cxr", name="cxr")
        nc.scalar.activation(cxr[:], cxp[:], AF.Copy, scale=rden[:, 0:1])
        attn_state[a]["cxr"] = cxr

    def attention_c(a):
        t0 = 4 * a
        cxr = attn_state[a]["cxr"]
        tpc = psA.tile([128, 64], f32, tag="a", name="tpc")
        for ub in range(4):
            nc.tensor.transpose(tpc[:, ub * 16:(ub + 1) * 16],
                                cxr[:, ub * 128:(ub + 1) * 128],
                                ident[0:16, 0:16])
        cxt = est.tile([128, 64], bf16, tag="cxt", name="cxt")
        nc.vector.tensor_copy(out=cxt[:], in_=tpc[:])
        sp = psA.tile([128, 64], f32, tag="a", name="sp")
        for mt in range(4):
            nc.tensor.matmul(sp[:, mt * 16:(mt + 1) * 16],
                             wcb_sb[0:1, mt * 128:(mt + 1) * 128],
                             ones_b[0:1, 0:16],
                             start=True, stop=False, skip_group_check=True)
            for kt in range(8):
                rhs = (cxt[:, kt * 16:(kt + 1) * 16] if kt < 4
                       else hist_v[:, t0 + 1:t0 + 5, kt - 4, 0:4]
                       .rearrange("p t b -> p b t"))
                nc.tensor.matmul(
                    sp[:, mt * 16:(mt + 1) * 16],
                    wc[:, (kt * 4 + mt) * 128:(kt * 4 + mt + 1) * 128],
                    rhs, start=False, stop=(kt == 7),
                    skip_group_check=True)
        nc.scalar.activation(sT_v[:, :, a, :], sp[:].rearrange(
            "p (m i) -> p m i", m=4), AF.Tanh)
        nc.sync.dma_start(
            d["ag_in"][a][:].rearrange("p (k i) -> p k i", k=4),
            sT_v[:, :, a, :])
        nc.gpsimd.collective_compute(
            "AllGather", mybir.AluOpType.bypass,
            replica_groups=[list(range(NCORES))],
            ins=[d["ag_in"][a][:]], outs=[d["ag_out"][a][:]])
        ago_v = d["ag_out"][a].rearrange("(j p) (k i) -> p k j i",
                                         j=NCORES, k=4)
        for kt in range(4):
            nc.sync.dma_start(sg_v5[:, a, kt], ago_v[:, kt])

    out_v = d["out"].rearrange("(j bl) (a tl) v -> a (j bl) tl v",
                               j=NCORES, a=NBLK)
    ob_state = {}

    def vocab_chunk(a, vc):
        ps = psV.tile([128, 512], f32, tag="v", name="vps")
        for kt in range(4):
            nc.tensor.matmul(
                ps[:, 0:VCH], sg_v[:, a, kt],
                wh_sb[:, kt * VS + vc * VCH: kt * VS + (vc + 1) * VCH],
                start=(kt == 0), stop=(kt == 3), skip_group_check=True)
        if vc % 2 == 0:
            ob = outp.tile([128, 2 * VCH], bf16, tag="ob", name="ob")
            ob_state[a] = ob
            nc.scalar.activation(ob[:, 0:VCH], ps[:, 0:VCH], AF.Copy)
        else:
            ob = ob_state[a]
            nc.vector.tensor_copy(out=ob[:, VCH:2 * VCH], in_=ps[:, 0:VCH])
            nc.sync.dma_start(
                out_v[a, :, :, (vc - 1) * VCH:(vc + 1) * VCH], ob[:])

    # ==== schedule ======================================================
    # prologue: embedding transposes for first half + xp(h2=0) so step 0
    # can start; everything else becomes filler between early steps.
    for c in range(4):
        emb_transpose(c)
    for mt in range(16):
        xproj(mt, 0)

    LOW = 10_000_000

    for t in range(LQ):
        lstm_step(t)
        with tc.high_priority(offset=-LOW):
            # pure filler at low scheduler priority: runs only when the
            # recurrence/attention chains leave an engine idle
            if t < 2:
                # remaining embedding transposes (must all land before any
                # h2=1 xproj reads xT's second half)
                emb_transpose(4 + 2 * t)
                emb_transpose(5 + 2 * t)
                keyproj(2 * t)
                keyproj(2 * t + 1)
            elif t < 6:
                # xp second half (tokens 16-31), needed from step 16
                xproj(4 * (t - 2) + 0, 1)
                xproj(4 * (t - 2) + 1, 1)
                xproj(4 * (t - 2) + 2, 1)
                xproj(4 * (t - 2) + 3, 1)
            # vocab: chunks 0-3 of block a at steps 4a+8..11, chunks 4-7 at
            # 4a+12..15 — each step touches two blocks so a late AllGather
            # on the newer block cannot starve the PE.
            if t >= 8 and (t - 8) // 4 < NBLK:
                vocab_chunk((t - 8) // 4, (t - 8) % 4)
            if t >= 12 and (t - 12) // 4 < NBLK:
                vocab_chunk((t - 12) // 4, 4 + (t - 12) % 4)
        if t >= 3 and (t - 3) % 4 == 0:
            attention_a((t - 3) // 4)
        if t >= 4 and (t - 4) % 4 == 0:
            attention_b((t - 4) // 4)
            attention_c((t - 4) // 4)
    # tail: attention for block 7 first (posts the last AllGather), then the
    # leftover vocab chunks (blocks 5-6 ready immediately, 7 after its AG)
    attention_b(7)
    attention_c(7)
    with tc.high_priority(offset=-LOW):
        for vc in range(4, NVC):
            vocab_chunk(5, vc)
        for vc in range(NVC):
            vocab_chunk(6, vc)
        for vc in range(NVC):
            vocab_chunk(7, vc)

    for pool in (outp, psV, psA, psG, big, est, perm):
        pool.release()


# ======================================================================
# host side
# ======================================================================

def _bf16(x):
    return np.ascontiguousarray(np.asarray(x, np.float32).astype(ml_dtypes.bfloat16))


def _tiles(wT, ktn, mtn):
    """[K, M] -> [128, ktn*mtn*128]; tile (kt,mt) at col (kt*mtn+mt)*128."""
    K, M = wT.shape
    assert K == ktn * 128 and M == mtn * 128
    t = wT.reshape(ktn, 128, mtn, 128).transpose(1, 0, 2, 3)
    return np.ascontiguousarray(t.reshape(128, ktn * mtn * 128))


def kernel(src_padding_mask, enc_hidden_states, enc_last_hidden_state,
           tgt_batch, sos_idx, emb, W_ih, W_hh, b_ih, b_hh, Wa_w, Wa_b,
           Ua_w, Ua_b, va_w, va_b, Wc_w, Wc_b, Wh_w, Wh_b):
    import concourse.bass_utils as bass_utils

    if "nc" not in _cache:
        _cache["nc"] = _build_program()
    nc = _cache["nc"]

    f32 = np.float32
    emb = np.ascontiguousarray(np.asarray(emb, f32))
    enc = np.asarray(enc_hidden_states, f32)
    h0 = np.asarray(enc_last_hidden_state, f32)[0]
    mask = np.asarray(src_padding_mask, np.int32)

    ids = np.empty((LQ, B), np.int64)
    ids[0, :] = int(sos_idx)
    ids[1:, :] = np.asarray(tgt_batch)[:, :-1].T

    # H = 2h scaling: 0.5 folded into W_hh / Wa / Wc-h-part; h0 stored as 2h0.
    # Additionally i/f/o gate rows (units 0:512, 512:1024, 1536:2048) carry an
    # extra 0.5 so sigmoid(x) = 0.5*(tanh(x/2)+1) needs no ACT scale.
    gsc = np.ones((4 * H, 1), f32)
    gsc[0:H] = 0.5          # i
    gsc[H:2 * H] = 0.5      # f
    gsc[3 * H:] = 0.5       # o
    whh_t = _bf16(_tiles((0.5 * gsc * np.asarray(W_hh, f32)).T, 4, 16))
    wih_t = _bf16(_tiles((gsc * np.asarray(W_ih, f32)).T, 2, 16))
    wa_t = _bf16(_tiles(0.5 * np.asarray(Wa_w, f32).T, 4, 4))
    wc_scaled = np.asarray(Wc_w, f32).copy()
    wc_scaled[:, H:] *= 0.5
    wc_t = _bf16(_tiles(wc_scaled.T, 8, 4))
    ua_t = _bf16(_tiles(np.asarray(Ua_w, f32).T, 4, 4))
    bias_units = np.ascontiguousarray(
        (gsc[:, 0] * (np.asarray(b_ih, f32) + np.asarray(b_hh, f32)))
        .reshape(16, 128).T)
    bkp = np.ascontiguousarray(
        (np.asarray(Ua_b, f32) + np.asarray(Wa_b, f32)).reshape(4, 128).T)
    va_c = _bf16(np.asarray(va_w, f32)[0].reshape(4, 128).T)
    wcb_row = _bf16(np.asarray(Wc_b, f32).reshape(1, 512))
    vab = float(np.asarray(va_b, f32).reshape(-1)[0])
    bdm = np.zeros((128, 4), f32)
    for p in range(128):
        bdm[p, p // 32] = 1.0
    bd16 = np.zeros((128, 16), f32)
    for p in range(128):
        for m in range(16):
            if m // 4 == p // 32:
                bd16[p, m] = 1.0
    m16 = np.zeros((16, 4), f32)
    for p in range(16):
        m16[p, p % 4] = 1.0
    WhT = _bf16(np.asarray(Wh_w, f32).T)
    Whb = np.asarray(Wh_b, f32)

    in_maps = []
    for j in range(NCORES):
        own = np.arange(4 * j, 4 * j + 4)
        permb = np.concatenate([own, np.setdiff1d(np.arange(B), own)])
        ids_p = ids[:, permb]
        idx = np.ascontiguousarray(
            ids_p.reshape(LQ * B).astype(np.int32).reshape(8, 128).T)
        h0p = 2.0 * h0[permb]
        h0t = np.zeros((128, 128), f32)
        for kt in range(4):
            h0t[:, kt * 32:(kt + 1) * 32] = h0p[:, kt * 128:(kt + 1) * 128].T
        enc_own = enc[own]                                   # [4, 32, 512]
        encr = _bf16(enc_own.reshape(128, 512))
        enctl = _bf16(
            enc_own.reshape(128, 4, 128).transpose(2, 1, 0).reshape(128, 512))
        maskbias = np.full((128, 1), vab, f32)
        mrows = mask[own].reshape(128)            # p = b*32 + l
        maskbias[mrows == 0, 0] = vab - 30.0
        in_maps.append({
            "emb": emb, "idx": idx, "whh": whh_t, "wih": wih_t, "wa": wa_t,
            "wc": wc_t, "ua": ua_t, "bias_units": bias_units, "bias_kp": bkp,
            "va": va_c, "wcb_row": wcb_row, "h0t": _bf16(h0t),
            "encr": encr, "enct": enctl,
            "maskbias": maskbias, "m16": m16, "bd16": _bf16(bd16),
            "bdm": _bf16(bdm),
            "wht": np.ascontiguousarray(WhT[:, j * VS:(j + 1) * VS]),
        })

    res = bass_utils.run_bass_kernel_spmd(nc, in_maps, list(range(NCORES)))
    out = np.concatenate(
        [np.asarray(res.results[jj]["out"]) for jj in range(NCORES)], axis=2)
    out = out.astype(np.float32)
    out += Whb[None, None, :]
    return np.ascontiguousarray(out)


if __name__ == "__main__":
    import reference
    inp = dict(reference.setup_inputs())
    got = kernel(**{k: (np.asarray(v) if hasattr(v, "shape") else v)
                    for k, v in inp.items()})
    print("out shape", got.shape, got.dtype)


# revision 13
# speedup vs baseline: 1.3300x; 1.3300x over previous
"""Trainium2 Bass kernel for nn_Decoder (LSTM decoder + additive attention + vocab head).

Sharding (8 NeuronCores, SPMD — identical program, per-core data):
  - LSTM h/c recurrence replicated on all cores, transposed layout (units on
    partitions, batch on free). Hidden state stored scaled H=2h so sigmoid
    gates fuse into scalar_tensor_tensor ops (0.5 folded into W_hh/Wa/Wc-h).
  - Attention + s_t = tanh(Wc @ [ctx; h]) sharded over batch (4/core) via a
    host-side batch permutation. Scores computed transposed ([128 (b,l), 4 t])
    with the padding mask + va_b folded into the Exp bias; softmax division
    deferred until after the enc contraction (per-partition rescale).
  - s gathered incrementally with 8 small AllGathers (overlap the recurrence);
    vocab projection sharded over V (4000 rows/core), chunks interleaved
    2-per-LSTM-step so the PE never idles. Wh_b is added on the host; the
    device writes bf16 logits.
Only Tanh/Exp ACT funcs are used (single table set).
"""

import numpy as np
import ml_dtypes

V, E, H, LQ, B = 32000, 256, 512, 32, 32
NCORES = 8
VS = V // NCORES      # 4000
VCH = 500             # vocab chunk (fp32 psum bank = 512)
NVC = VS // VCH       # 8
NBLK = LQ // 4        # 8 four-step attention blocks

_cache = {}


def _build_program():
    import concourse.bass as bass
    import concourse.mybir as mybir
    import concourse.tile as tile
    from concourse import bacc

    f32 = mybir.dt.float32
    bf16 = mybir.dt.bfloat16
    i32 = mybir.dt.int32

    nc = bacc.Bacc("TRN2", target_bir_lowering=False, debug=False,
                   num_devices=NCORES)

    d = {}

    def inp(name, shape, dtype):
        d[name] = nc.dram_tensor(name, shape, dtype, kind="ExternalInput").ap()

    inp("emb", [V, E], f32)
    inp("idx", [128, 8], i32)
    inp("whh", [128, 64 * 128], bf16)
    inp("wih", [128, 32 * 128], bf16)
    inp("wa", [128, 16 * 128], bf16)
    inp("wc", [128, 32 * 128], bf16)
    inp("ua", [128, 16 * 128], bf16)
    inp("bias_units", [128, 16], f32)
    inp("bias_kp", [128, 4], f32)
    inp("va", [128, 4], bf16)
    inp("wcb_row", [1, 512], bf16)
    inp("h0t", [128, 128], bf16)
    inp("encr", [128, 512], bf16)
    inp("enct", [128, 512], bf16)
    inp("maskbias", [128, 1], f32)
    inp("m16", [16, 4], f32)
    inp("bd16", [128, 16], bf16)
    inp("bdm", [128, 4], bf16)
    inp("wht", [512, VS], bf16)
    d["out"] = nc.dram_tensor("out", [B, LQ, VS], bf16, kind="ExternalOutput").ap()
    d["ag_in"] = [nc.dram_tensor(f"agi{a}", [128, 64], bf16).ap()
                  for a in range(NBLK)]
    d["ag_out"] = [nc.dram_tensor(f"ago{a}", [128 * NCORES, 64], bf16,
                                  addr_space="Shared").ap() for a in range(NBLK)]

    with tile.TileContext(nc) as tc:
        _emit(tc, bass, mybir, d)
    nc.compile()
    return nc


def _emit(tc, bass, mybir, d):
    from concourse.masks import make_identity

    f32 = mybir.dt.float32
    bf16 = mybir.dt.bfloat16
    i32 = mybir.dt.int32
    AF = mybir.ActivationFunctionType
    OP = mybir.AluOpType
    nc = tc.nc

    perm = tc.alloc_tile_pool(name="perm", bufs=1)
    est = tc.alloc_tile_pool(name="est", bufs=2)
    big = tc.alloc_tile_pool(name="big", bufs=1)
    psG = tc.alloc_tile_pool(name="psG", bufs=1, space="PSUM")
    psA = tc.alloc_tile_pool(name="psA", bufs=3, space="PSUM")
    psV = tc.alloc_tile_pool(name="psV", bufs=3, space="PSUM")
    outp = tc.alloc_tile_pool(name="outp", bufs=3)

    def pt(name, shape, dtype):
        return perm.tile(shape, dtype, tag=name, name=name)

    # ---- persistent SBUF ----------------------------------------------
    whh = pt("whh", [128, 64 * 128], bf16)   # lhsT tile (kt,mt) @ (kt*16+mt)*128
    wih = pt("wih", [128, 32 * 128], bf16)   # (kt*16+mt)*128, kt<2
    wa = pt("wa", [128, 16 * 128], bf16)     # (kt*4+mt)*128
    wc = pt("wc", [128, 32 * 128], bf16)     # (kt*4+mt)*128, kt<8
    ua = pt("ua", [128, 16 * 128], bf16)
    bias_u = pt("bias_u", [128, 16], f32)
    bias_kp = pt("bias_kp", [128, 4], f32)
    va_sb = pt("va_sb", [128, 4], bf16)
    wcb_sb = pt("wcb_sb", [1, 512], bf16)
    idx_sb = pt("idx_sb", [128, 8], i32)
    encr = pt("encr", [128, 512], bf16)      # rows (b_own,l), cols u
    enct = pt("enct", [128, 512], bf16)      # cols (kt, (b_own,l))
    mbias = pt("mbias", [128, 1], f32)       # va_b + (0 | -30) per (b_own,l)
    m16_sb = pt("m16_sb", [16, 4], f32)
    bd16_sb = pt("bd16_sb", [128, 16], bf16)
    bdm = pt("bdm", [128, 4], bf16)
    hist = pt("hist", [128, 33 * 128], bf16)  # col = slot*128 + kt*32 + b; H=2h
    cS = pt("cS", [128, 128], f32)            # S = 2c
    xpb = pt("xpb", [128, 16 * 1024], bf16)  # col = mt*1024 + t*32 + b
    xT = pt("xT", [128, 2 * 1024], bf16)     # col = kt*1024 + (t*32+b)
    kp = pt("kp", [128, 512], f32)           # col = mt*128 + b*32 + l
    sT = pt("sT", [128, 512], bf16)           # col = mt*128 + a*16 + (b*4+t)
    wh_sb = pt("wh_sb", [128, 4 * VS], bf16)  # col = kt*VS + v
    ident = pt("ident", [128, 128], f32)
    ident_b = pt("ident_b", [128, 128], bf16)
    ones_b = pt("ones_b", [1, 128], bf16)
    sg = pt("sg", [128, NBLK * 4 * 128], bf16)  # col = a*512+kt*128+(j2*16+b*4+t)

    hist_v = hist[:].rearrange("p (s k b) -> p s k b", s=33, k=4)
    xpb_v = xpb[:].rearrange("p (m t) -> p m t", m=16)
    kp_v = kp[:].rearrange("p (m b l) -> p m b l", m=4, b=4)
    sT_v = sT[:].rearrange("p (k a i) -> p k a i", k=4, a=NBLK)
    sg_v = sg[:].rearrange("p (a k i) -> p a k i", a=NBLK, k=4)
    sg_v5 = sg[:].rearrange("p (a k j i) -> p a k j i", a=NBLK, k=4, j=NCORES)

    # ---- load weights/constants ---------------------------------------
    # step-0-critical loads first; the big vocab weights (needed from t=8)
    # last so their transfers don't delay the recurrence start
    for dst, src in ((idx_sb, "idx"), (wih, "wih"), (whh, "whh"),
                     (bias_u, "bias_units"), (ua, "ua"), (enct, "enct"),
                     (bias_kp, "bias_kp"), (wa, "wa"), (wc, "wc"),
                     (encr, "encr"), (va_sb, "va"), (wcb_sb, "wcb_row"),
                     (mbias, "maskbias"), (m16_sb, "m16"),
                     (bd16_sb, "bd16"), (bdm, "bdm")):
        nc.sync.dma_start(dst[:], d[src][:])
    nc.sync.dma_start(hist[:, 0:128], d["h0t"][:])
    for kt in range(4):
        nc.sync.dma_start(wh_sb[:, kt * VS:(kt + 1) * VS],
                          d["wht"][kt * 128:(kt + 1) * 128, :])

    make_identity(nc, ident[:])
    make_identity(nc, ident_b[:])
    nc.gpsimd.memset(ones_b[:], 1.0)
    nc.gpsimd.memset(cS[:], 0.0)

    # ---- embedding gather + transpose ---------------------------------
    xrows = []
    for c in range(8):
        xrow = big.tile([128, 256], f32, tag=f"xrow{c}", name=f"xrow{c}")
        nc.gpsimd.indirect_dma_start(
            out=xrow[:], out_offset=None, in_=d["emb"][:],
            in_offset=bass.IndirectOffsetOnAxis(ap=idx_sb[:, c:c + 1], axis=0))
        xrows.append(xrow)

    def emb_transpose(c):
        for kt in range(2):
            tp = psA.tile([128, 128], f32, tag="a", name="tpx")
            nc.tensor.transpose(tp[:], xrows[c][:, kt * 128:(kt + 1) * 128],
                                ident[:, 0:128])
            seg = xT[:, kt * 1024 + c * 128: kt * 1024 + (c + 1) * 128]
            if (c + kt) % 2 == 0:
                nc.vector.tensor_copy(out=seg, in_=tp[:])
            else:
                nc.scalar.activation(seg, tp[:], AF.Copy)

    # ---- x-projection xpb = W_ih @ x^T + (b_ih + b_hh), bf16 ----------
    def xproj(mt, h2):
        ps = psV.tile([128, 512], f32, tag="v", name="vps")
        for kt in range(2):
            nc.tensor.matmul(
                ps[:],
                wih[:, (kt * 16 + mt) * 128:(kt * 16 + mt + 1) * 128],
                xT[:, kt * 1024 + h2 * 512: kt * 1024 + (h2 + 1) * 512],
                start=(kt == 0), stop=(kt == 1))
        dst = xpb[:, mt * 1024 + h2 * 512: mt * 1024 + (h2 + 1) * 512]
        if (mt + h2) % 2 == 0:
            nc.vector.tensor_scalar(out=dst, in0=ps[:],
                                    scalar1=bias_u[:, mt:mt + 1],
                                    scalar2=None, op0=OP.add)
        else:
            nc.scalar.activation(dst, ps[:], AF.Identity,
                                 bias=bias_u[:, mt:mt + 1])

    # ---- key projection kp = Ua @ enc^T + (Ua_b + Wa_b) ---------------
    def keyproj(mt):
        ps = psA.tile([128, 128], f32, tag="a", name="kp_ps")
        for kt in range(4):
            nc.tensor.matmul(
                ps[:], ua[:, (kt * 4 + mt) * 128:(kt * 4 + mt + 1) * 128],
                enct[:, kt * 128:(kt + 1) * 128],
                start=(kt == 0), stop=(kt == 3))
        nc.vector.tensor_scalar(out=kp[:, mt * 128:(mt + 1) * 128], in0=ps[:],
                                scalar1=bias_kp[:, mt:mt + 1], scalar2=None,
                                op0=OP.add)

    # ==== per-step bodies ==============================================
    def lstm_step(t):
        # Gfg = [f-gates | g-gates] (mt 4..11); Gio = [i | o] (mt 0..3,12..15)
        Gfg = psG.tile([128, 256], f32, tag="gfg", name="Gfg")
        Gio = psG.tile([128, 256], f32, tag="gio", name="Gio")
        nc.tensor.matmul(Gfg[:], ident_b[:], xpb_v[:, 4:12, t * 32:(t + 1) * 32],
                         start=True, stop=False, skip_group_check=True)
        nc.tensor.matmul(Gio[:, 0:128], ident_b[:],
                         xpb_v[:, 0:4, t * 32:(t + 1) * 32],
                         start=True, stop=False, skip_group_check=True)
        nc.tensor.matmul(Gio[:, 128:256], ident_b[:],
                         xpb_v[:, 12:16, t * 32:(t + 1) * 32],
                         start=True, stop=False, skip_group_check=True)

        def gmm(dst, col, mt):
            for kt in range(4):
                nc.tensor.matmul(
                    dst[:, col * 32:(col + 1) * 32],
                    whh[:, (kt * 16 + mt) * 128:(kt * 16 + mt + 1) * 128],
                    hist_v[:, t, kt, :], start=False, stop=(kt == 3),
                    skip_group_check=True)
        for mt in range(4, 12):        # f then g
            gmm(Gfg, mt - 4, mt)
        for mt in range(4):            # i
            gmm(Gio, mt, mt)
        for mt in range(12, 16):       # o
            gmm(Gio, mt - 8, mt)
        tf_ = est.tile([128, 128], f32, tag="tf", name="tf_")
        tg = est.tile([128, 128], f32, tag="tg", name="tg")
        ti = est.tile([128, 128], f32, tag="ti", name="ti")
        to_ = est.tile([128, 128], f32, tag="to", name="to_")
        nc.scalar.activation(tf_[:], Gfg[:, 0:128], AF.Tanh)
        nc.scalar.activation(tg[:], Gfg[:, 128:256], AF.Tanh)
        nc.scalar.activation(ti[:], Gio[:, 0:128], AF.Tanh)
        nc.scalar.activation(to_[:], Gio[:, 128:256], AF.Tanh)
        # S' = 0.5*(t_f+1)*S + (t_i+1)*g   (S = 2c)
        P = est.tile([128, 128], f32, tag="P", name="P")
        Bt = est.tile([128, 128], f32, tag="Bt", name="Bt")
        nc.vector.scalar_tensor_tensor(out=P[:], in0=tf_[:],
                                       scalar=1.0, in1=cS[:],
                                       op0=OP.add, op1=OP.mult)
        nc.vector.scalar_tensor_tensor(out=Bt[:], in0=ti[:],
                                       scalar=1.0, in1=tg[:],
                                       op0=OP.add, op1=OP.mult)
        nc.vector.scalar_tensor_tensor(out=cS[:], in0=P[:], scalar=0.5,
                                       in1=Bt[:], op0=OP.mult, op1=OP.add)
        tct = est.tile([128, 128], f32, tag="tct", name="tct")
        nc.scalar.activation(tct[:], cS[:], AF.Tanh, scale=0.5)
        # H = (t_o+1)*tanh(c) = 2h
        nc.vector.scalar_tensor_tensor(
            out=hist[:, (t + 1) * 128:(t + 2) * 128], in0=to_[:],
            scalar=1.0, in1=tct[:], op0=OP.add, op1=OP.mult)

    attn_state = {}

    def attention_a(a):
        t0 = 4 * a
        qp = psA.tile([128, 4, 4, 4], f32, tag="a", name="qp")  # (mt, t, b)
        for mt in range(4):
            for kt in range(4):
                nc.tensor.matmul(
                    qp[:, mt, :, :],
                    wa[:, (kt * 4 + mt) * 128:(kt * 4 + mt + 1) * 128],
                    hist_v[:, t0 + 1:t0 + 5, kt, 0:4],
                    start=(kt == 0), stop=(kt == 3))
        tin = big.tile([128, 4, 4, 4, 32], f32, tag="tin", name="tin")
        for mt in range(4):
            nc.vector.tensor_tensor(
                out=tin[:, mt],
                in0=qp[:, mt].unsqueeze(3).to_broadcast([128, 4, 4, 32]),
                in1=kp_v[:, mt].unsqueeze(1).to_broadcast([128, 4, 4, 32]),
                op=OP.add)
        tnh = big.tile([128, 2048], bf16, tag="tnh", name="tnh")
        for mt in range(4):
            nc.scalar.activation(
                tnh[:, mt * 512:(mt + 1) * 512],
                tin[:, mt].rearrange("p a b l -> p (a b l)"), AF.Tanh)
        attn_state[a] = dict(tnh=tnh)

    def attention_b(a):
        tnh = attn_state[a]["tnh"]
        # scores transposed: escT[(b,l), t], contracted over u via tnh-as-lhsT
        escT = psA.tile([128, 4], f32, tag="a", name="escT")
        for t in range(4):
            for mt in range(4):
                nc.tensor.matmul(
                    escT[:, t:t + 1],
                    tnh[:, mt * 512 + t * 128: mt * 512 + (t + 1) * 128],
                    va_sb[:, mt:mt + 1],
                    start=(mt == 0), stop=(mt == 3), skip_group_check=True)
        # esc = exp(score + va_b + mask*-30)  (unnormalized)
        esc4 = est.tile([128, 4], bf16, tag="esc4", name="esc4")
        nc.scalar.activation(esc4[:], escT[:], AF.Exp, bias=mbias[:, 0:1])
        # den16[(b,t)] = sum_l esc[(b,l), t]
        den16 = psA.tile([16, 4], f32, tag="a", name="den16")
        nc.tensor.matmul(den16[:], bd16_sb[:], esc4[:], start=True, stop=True)
        rec16 = est.tile([16, 4], f32, tag="rec16", name="rec16")
        nc.vector.reciprocal(rec16[:], den16[:])
        rm = est.tile([16, 4], f32, tag="rm", name="rm")
        nc.vector.tensor_tensor(out=rm[:], in0=rec16[:], in1=m16_sb[:],
                                op=OP.mult)
        rden = est.tile([16, 1], f32, tag="rden", name="rden")
        nc.vector.tensor_reduce(out=rden[:], in_=rm[:],
                                axis=mybir.AxisListType.X, op=OP.add)
        # block-diag weights A[(b,l), (b',t)] = esc[(b,l), t] * [b'==b]
        abig = est.tile([128, 4, 4], bf16, tag="abig", name="abig")
        nc.vector.tensor_tensor(
            out=abig[:], in0=bdm[:].unsqueeze(2).to_broadcast([128, 4, 4]),
            in1=esc4[:].unsqueeze(1).to_broadcast([128, 4, 4]), op=OP.mult)
        cxp = psA.tile([16, 512], f32, tag="a", name="cxp")
        nc.tensor.matmul(cxp[:],
                         abig[:].rearrange("p b t -> p (b t)"),
                         encr[:], start=True, stop=True)
        # normalize deferred: ctx = cxp * (1/den) per partition (b,t)
        cxr = est.tile([16, 512], f32, tag="cxr", name="cxr")
        nc.scalar.activation(cxr[:], cxp[:], AF.Copy, scale=rden[:, 0:1])
        attn_state[a]["cxr"] = cxr

    def attention_c(a):
        t0 = 4 * a
        cxr = attn_state[a]["cxr"]
        tpc = psA.tile([128, 64], f32, tag="a", name="tpc")
        for ub in range(4):
            nc.tensor.transpose(tpc[:, ub * 16:(ub + 1) * 16],
                                cxr[:, ub * 128:(ub + 1) * 128],
                                ident[0:16, 0:16])
        cxt = est.tile([128, 64], bf16, tag="cxt", name="cxt")
        nc.vector.tensor_copy(out=cxt[:], in_=tpc[:])
        sp = psA.tile([128, 64], f32, tag="a", name="sp")
        for mt in range(4):
            nc.tensor.matmul(sp[:, mt * 16:(mt + 1) * 16],
                             wcb_sb[0:1, mt * 128:(mt + 1) * 128],
                             ones_b[0:1, 0:16],
                             start=True, stop=False, skip_group_check=True)
            for kt in range(8):
                rhs = (cxt[:, kt * 16:(kt + 1) * 16] if kt < 4
                       else hist_v[:, t0 + 1:t0 + 5, kt - 4, 0:4]
                       .rearrange("p t b -> p b t"))
                nc.tensor.matmul(
                    sp[:, mt * 16:(mt + 1) * 16],
                    wc[:, (kt * 4 + mt) * 128:(kt * 4 + mt + 1) * 128],
                    rhs, start=False, stop=(kt == 7),
                    skip_group_check=True)
        nc.scalar.activation(sT_v[:, :, a, :], sp[:].rearrange(
            "p (m i) -> p m i", m=4), AF.Tanh)
        nc.sync.dma_start(
            d["ag_in"][a][:].rearrange("p (k i) -> p k i", k=4),
            sT_v[:, :, a, :])
        nc.gpsimd.collective_compute(
            "AllGather", mybir.AluOpType.bypass,
            replica_groups=[list(range(NCORES))],
            ins=[d["ag_in"][a][:]], outs=[d["ag_out"][a][:]])
        ago_v = d["ag_out"][a].rearrange("(j p) (k i) -> p k j i",
                                         j=NCORES, k=4)
        for kt in range(4):
            nc.sync.dma_start(sg_v5[:, a, kt], ago_v[:, kt])

    out_v = d["out"].rearrange("(j bl) (a tl) v -> a (j bl) tl v",
                               j=NCORES, a=NBLK)
    ob_state = {}

    def vocab_chunk(a, vc):
        ps = psV.tile([128, 512], f32, tag="v", name="vps")
        for kt in range(4):
            nc.tensor.matmul(
                ps[:, 0:VCH], sg_v[:, a, kt],
                wh_sb[:, kt * VS + vc * VCH: kt * VS + (vc + 1) * VCH],
                start=(kt == 0), stop=(kt == 3), skip_group_check=True)
        if vc % 2 == 0:
            ob = outp.tile([128, 2 * VCH], bf16, tag="ob", name="ob")
            ob_state[a] = ob
            nc.scalar.activation(ob[:, 0:VCH], ps[:, 0:VCH], AF.Copy)
        else:
            ob = ob_state[a]
            nc.vector.tensor_copy(out=ob[:, VCH:2 * VCH], in_=ps[:, 0:VCH])
            nc.sync.dma_start(
                out_v[a, :, :, (vc - 1) * VCH:(vc + 1) * VCH], ob[:])

    # ==== schedule ======================================================
    # prologue: embedding transposes for first half + xp(h2=0) so step 0
    # can start; everything else becomes filler between early steps.
    for c in range(4):
        emb_transpose(c)
    for mt in range(16):
        xproj(mt, 0)

    LOW = 10_000_000

    for t in range(LQ):
        lstm_step(t)
        with tc.high_priority(offset=-LOW):
            # pure filler at low scheduler priority: runs only when the
            # recurrence/attention chains leave an engine idle
            if t < 2:
                # remaining embedding transposes (must all land before any
                # h2=1 xproj reads xT's second half)
                emb_transpose(4 + 2 * t)
                emb_transpose(5 + 2 * t)
                keyproj(2 * t)
                keyproj(2 * t + 1)
            elif t < 6:
                # xp second half (tokens 16-31), needed from step 16
                xproj(4 * (t - 2) + 0, 1)
                xproj(4 * (t - 2) + 1, 1)
                xproj(4 * (t - 2) + 2, 1)
                xproj(4 * (t - 2) + 3, 1)
            # vocab: chunks 0-3 of block a at steps 4a+8..11, chunks 4-7 at
            # 4a+12..15 — each step touches two blocks so a late AllGather
            # on the newer block cannot starve the PE.
            if t >= 8 and (t - 8) // 4 < NBLK:
                vocab_chunk((t - 8) // 4, (t - 8) % 4)
            if t >= 12 and (t - 12) // 4 < NBLK:
                vocab_chunk((t - 12) // 4, 4 + (t - 12) % 4)
        if t >= 3 and (t - 3) % 4 == 0:
            attention_a((t - 3) // 4)
        if t >= 4 and (t - 4) % 4 == 0:
            attention_b((t - 4) // 4)
            attention_c((t - 4) // 4)
    # tail: attention for block 7 first (posts the last AllGather), then the
    # leftover vocab chunks (blocks 5-6 ready immediately, 7 after its AG)
    attention_b(7)
    attention_c(7)
    with tc.high_priority(offset=-LOW):
        for vc in range(4, NVC):
            vocab_chunk(5, vc)
        for vc in range(NVC):
            vocab_chunk(6, vc)
        for vc in range(NVC):
            vocab_chunk(7, vc)

    for pool in (outp, psV, psA, psG, big, est, perm):
        pool.release()


# ======================================================================
# host side
# ======================================================================

def _bf16(x):
    return np.ascontiguousarray(np.asarray(x, np.float32).astype(ml_dtypes.bfloat16))


def _tiles(wT, ktn, mtn):
    """[K, M] -> [128, ktn*mtn*128]; tile (kt,mt) at col (kt*mtn+mt)*128."""
    K, M = wT.shape
    assert K == ktn * 128 and M == mtn * 128
    t = wT.reshape(ktn, 128, mtn, 128).transpose(1, 0, 2, 3)
    return np.ascontiguousarray(t.reshape(128, ktn * mtn * 128))


def kernel(src_padding_mask, enc_hidden_states, enc_last_hidden_state,
           tgt_batch, sos_idx, emb, W_ih, W_hh, b_ih, b_hh, Wa_w, Wa_b,
           Ua_w, Ua_b, va_w, va_b, Wc_w, Wc_b, Wh_w, Wh_b):
    import concourse.bass_utils as bass_utils

    if "nc" not in _cache:
        _cache["nc"] = _build_program()
    nc = _cache["nc"]

    f32 = np.float32
    emb = np.ascontiguousarray(np.asarray(emb, f32))
    enc = np.asarray(enc_hidden_states, f32)
    h0 = np.asarray(enc_last_hidden_state, f32)[0]
    mask = np.asarray(src_padding_mask, np.int32)

    ids = np.empty((LQ, B), np.int64)
    ids[0, :] = int(sos_idx)
    ids[1:, :] = np.asarray(tgt_batch)[:, :-1].T

    # H = 2h scaling: 0.5 folded into W_hh / Wa / Wc-h-part; h0 stored as 2h0.
    # Additionally i/f/o gate rows (units 0:512, 512:1024, 1536:2048) carry an
    # extra 0.5 so sigmoid(x) = 0.5*(tanh(x/2)+1) needs no ACT scale.
    gsc = np.ones((4 * H, 1), f32)
    gsc[0:H] = 0.5          # i
    gsc[H:2 * H] = 0.5      # f
    gsc[3 * H:] = 0.5       # o
    whh_t = _bf16(_tiles((0.5 * gsc * np.asarray(W_hh, f32)).T, 4, 16))
    wih_t = _bf16(_tiles((gsc * np.asarray(W_ih, f32)).T, 2, 16))
    wa_t = _bf16(_tiles(0.5 * np.asarray(Wa_w, f32).T, 4, 4))
    wc_scaled = np.asarray(Wc_w, f32).copy()
    wc_scaled[:, H:] *= 0.5
    wc_t = _bf16(_tiles(wc_scaled.T, 8, 4))
    ua_t = _bf16(_tiles(np.asarray(Ua_w, f32).T, 4, 4))
    bias_units = np.ascontiguousarray(
        (gsc[:, 0] * (np.asarray(b_ih, f32) + np.asarray(b_hh, f32)))
        .reshape(16, 128).T)
    bkp = np.ascontiguousarray(
        (np.asarray(Ua_b, f32) + np.asarray(Wa_b, f32)).reshape(4, 128).T)
    va_c = _bf16(np.asarray(va_w, f32)[0].reshape(4, 128).T)
    wcb_row = _bf16(np.asarray(Wc_b, f32).reshape(1, 512))
    vab = float(np.asarray(va_b, f32).reshape(-1)[0])
    bdm = np.zeros((128, 4), f32)
    for p in range(128):
        bdm[p, p // 32] = 1.0
    bd16 = np.zeros((128, 16), f32)
    for p in range(128):
        for m in range(16):
            if m // 4 == p // 32:
                bd16[p, m] = 1.0
    m16 = np.zeros((16, 4), f32)
    for p in range(16):
        m16[p, p % 4] = 1.0
    WhT = _bf16(np.asarray(Wh_w, f32).T)
    Whb = np.asarray(Wh_b, f32)

    in_maps = []
    for j in range(NCORES):
        own = np.arange(4 * j, 4 * j + 4)
        permb = np.concatenate([own, np.setdiff1d(np.arange(B), own)])
        ids_p = ids[:, permb]
        idx = np.ascontiguousarray(
            ids_p.reshape(LQ * B).astype(np.int32).reshape(8, 128).T)
        h0p = 2.0 * h0[permb]
        h0t = np.zeros((128, 128), f32)
        for kt in range(4):
            h0t[:, kt * 32:(kt + 1) * 32] = h0p[:, kt * 128:(kt + 1) * 128].T
        enc_own = enc[own]                                   # [4, 32, 512]
        encr = _bf16(enc_own.reshape(128, 512))
        enctl = _bf16(
            enc_own.reshape(128, 4, 128).transpose(2, 1, 0).reshape(128, 512))
        maskbias = np.full((128, 1), vab, f32)
        mrows = mask[own].reshape(128)            # p = b*32 + l
        maskbias[mrows == 0, 0] = vab - 30.0
        in_maps.append({
            "emb": emb, "idx": idx, "whh": whh_t, "wih": wih_t, "wa": wa_t,
            "wc": wc_t, "ua": ua_t, "bias_units": bias_units, "bias_kp": bkp,
            "va": va_c, "wcb_row": wcb_row, "h0t": _bf16(h0t),
            "encr": encr, "enct": enctl,
            "maskbias": maskbias, "m16": m16, "bd16": _bf16(bd16),
            "bdm": _bf16(bdm),
            "wht": np.ascontiguousarray(WhT[:, j * VS:(j + 1) * VS]),
        })

    res = bass_utils.run_bass_kernel_spmd(nc, in_maps, list(range(NCORES)))
    out = np.concatenate(
        [np.asarray(res.results[jj]["out"]) for jj in range(NCORES)], axis=2)
    out = out.astype(np.float32)
    out += Whb[None, None, :]
    return np.ascontiguousarray(out)


if __name__ == "__main__":
    import reference
    inp = dict(reference.setup_inputs())
    got = kernel(**{k: (np.asarray(v) if hasattr(v, "shape") else v)
                    for k, v in inp.items()})
    print("out shape", got.shape, got.dtype)
